# revision 69
# baseline (speedup 1.0000x reference)
"""Trainium2 Bass kernel for nn_MoEFusion (multi-modal MoE fusion MLP).

Data-parallel across 8 NeuronCores: batch dim (32768) sharded into 8
slices of 4096, all weights (<1 MB) replicated. No collectives.

The v1 all-bf16 pipeline structure with a surgical fp8 x-path:
  - features DMA'd as fp8e4m3 (halves HBM traffic vs bf16)
  - proj: 9 DoubleRow fp8 passes per stripe (vs 18 bf16)
  - x evicted as fp8 (weights pre-scaled x32 against e4m3 denormals;
    the ACT eviction scale divides it back out)
  - gate: 1 DoubleRow + 1 plain fp8 pass (vs 3 bf16)
  - W1: DoubleRow(k0,k1) + plain(k2) fp8 per expert = 16 passes (vs 24)
  - everything downstream of h (gating softmax path, broadcast, gating
    muls, W2/b2, pre, head) stays bf16 exactly as v1: DVE/GPSIMD
    elementwise runs 2x on 2-byte dtypes but half-rate on fp8, and
    bf16 passes interleave with DoubleRow passes at full PE rate.
  - eT bf16 (vs f32r) so the softmax colsum streams 1 col/cycle.
Measured numerics: rel err ~2.9e-3 vs threshold 2e-2.

On-device dataflow per core (feature-major "T" layout):
  featT [3, 768, 4096] fp8 --DMA--> SBUF per 512-col stripe
  xT = concat_m(projT_m + proj_b)  [128x3, 512] fp8
  gateT = exp(gate.T x + b), colsum via ones-matmul, reciprocal,
  gwT = eT * rsum^-1 (bf16); gw rows gathered to partition 0 (DMA),
  broadcast to 128 partitions on GPSIMD; sh_e = h_e * gw_e (DVE)
  fusedT = b2.T gwT + sum_e W2_e.T sh_e  (one PSUM accumulation)
  penT = relu(pre.T fused + pre_b); outT = head.T pen + head_b

Software pipeline: stage-2 (l2 accumulation) of stripe s-3 and
pre/head of older stripes are emitted during stripe s so the PE
stream never waits on the gate-softmax/broadcast chain.
"""

import sys

if "/opt/trn_rl_repo" not in sys.path:
    sys.path.insert(0, "/opt/trn_rl_repo")

from contextlib import ExitStack

import ml_dtypes
import numpy as np

# ---- problem constants (hardcoded per contract) ----
B = 32768
NCORES = 8
BL = B // NCORES  # 4096 per core
STRIPE = 512
NM = 3
NE = 8
D_IN = 768
KIN = D_IN // 128  # 6
D_P = 128
D_X = 384
KX = D_X // 128  # 3

BF16 = ml_dtypes.bfloat16
E4M3 = ml_dtypes.float8_e4m3

WS = 32.0   # fp8 weight pre-scale (e4m3 denormal avoidance)

# ---- fp8 packed weights (columns of [128, W8COLS]) ----
# W1 per expert is [k0|k1|k2|bias] blocks of 128 cols: the bias block
# (row 0 = WS*b1_e) rides in the second half of a DoubleRow pass whose
# moving block is the all-ones chunk of x, so h evictions need no ACT
# bias and can merge across expert pairs.
OFF_PROJ = 0                           # [p, m*768 + k*128 + o] = WS*proj_w
OFF_W1 = OFF_PROJ + NM * KIN * 128     # 2304: [p, e*512 + k*128 + o]
OFF_GATE = OFF_W1 + NE * 4 * 128       # 6400: [p, k*128 + e] (128-padded
W8COLS = OFF_GATE + KX * 128           # 6784   for DoubleRow stride rule)

# ---- bf16 packed weights ----
OFF_PRE = 0                            # [p, 0:64] = pre_w
OFF_HEAD = OFF_PRE + 64                # [p<64, 64:66] = head_w
OFF_ONES = OFF_HEAD + 2                # [p<8, 66:74] = 1.0 (colsum)
OFF_W2B = OFF_ONES + NE                # [p, 74 + e*128 + o] = w2[e, p, o]
OFF_B2B = OFF_W2B + NE * 128           # [p<8, o] = exp_b2[p, o]
WBFCOLS = OFF_B2B + 128                # 1226

# ---- f32 biases (columns of [128, WBCOLS]) ----
OFF_PROJB = 0
OFF_B1 = OFF_PROJB + NM
OFF_GATEB = OFF_B1 + NE
OFF_PREB = OFF_GATEB + 1
OFF_HEADB = OFF_PREB + 1
WBCOLS = OFF_HEADB + 1                 # 14


def pack_weights(inp):
    w8 = np.zeros((128, W8COLS), np.float32)
    pw = np.asarray(inp["proj_w"], np.float32) * WS
    w8[:, OFF_PROJ:OFF_W1] = (
        pw.reshape(NM, KIN, 128, 128).transpose(2, 0, 1, 3).reshape(128, -1)
    )
    w1 = np.asarray(inp["exp_w1"], np.float32) * WS
    blk = np.zeros((128, NE, 4, 128), np.float32)
    blk[:, :, :KX, :] = w1.reshape(NE, KX, 128, 128).transpose(2, 0, 1, 3)
    blk[0, :, KX, :] = np.asarray(inp["exp_b1"], np.float32) * WS
    w8[:, OFF_W1:OFF_GATE] = blk.reshape(128, -1)
    gw = np.asarray(inp["gate_w"], np.float32) * WS
    gblk = np.zeros((128, KX, 128), np.float32)
    gblk[:, :, :NE] = gw.reshape(KX, 128, NE).transpose(1, 0, 2)
    w8[:, OFF_GATE:W8COLS] = gblk.reshape(128, -1)
    w8 = w8.astype(E4M3)

    wbf = np.zeros((128, WBFCOLS), np.float32)
    wbf[:, OFF_PRE:OFF_HEAD] = np.asarray(inp["pre_w"], np.float32)
    wbf[:64, OFF_HEAD:OFF_ONES] = np.asarray(inp["head_w"], np.float32)
    wbf[:NE, OFF_ONES:OFF_W2B] = 1.0
    w2 = np.asarray(inp["exp_w2"], np.float32)
    wbf[:, OFF_W2B:OFF_B2B] = w2.transpose(1, 0, 2).reshape(128, -1)
    wbf[:NE, OFF_B2B:WBFCOLS] = np.asarray(inp["exp_b2"], np.float32)
    wbf = wbf.astype(BF16)

    wbias = np.zeros((128, WBCOLS), np.float32)
    wbias[:, OFF_PROJB:OFF_B1] = np.asarray(inp["proj_b"], np.float32).T
    wbias[:, OFF_B1:OFF_GATEB] = np.asarray(inp["exp_b1"], np.float32).T
    wbias[:NE, OFF_GATEB] = np.asarray(inp["gate_b"], np.float32)
    wbias[:64, OFF_PREB] = np.asarray(inp["pre_b"], np.float32)
    wbias[:2, OFF_HEADB] = np.asarray(inp["head_b"], np.float32)
    return w8, wbf, wbias


def build_program(n_stripes=BL // STRIPE):
    """Build the per-core Bass program (identical on all cores)."""
    import concourse.bacc as bacc
    import concourse.mybir as mybir
    import concourse.tile as tile

    f32 = mybir.dt.float32
    bf16 = mybir.dt.bfloat16
    fp8 = mybir.dt.float8e4
    AF = mybir.ActivationFunctionType
    DR = mybir.MatmulPerfMode.DoubleRow
    ALU = mybir.AluOpType
    bl = n_stripes * STRIPE

    nc = bacc.Bacc(
        "TRN2",
        target_bir_lowering=False,
        debug=False,
        enable_asserts=False,
    )

    featT = nc.dram_tensor("featT", [NM, D_IN, bl], fp8, kind="ExternalInput").ap()
    wmat8 = nc.dram_tensor("wmat8", [128, W8COLS], fp8, kind="ExternalInput").ap()
    wmatbf = nc.dram_tensor("wmatbf", [128, WBFCOLS], bf16, kind="ExternalInput").ap()
    wbias = nc.dram_tensor("wbias", [128, WBCOLS], f32, kind="ExternalInput").ap()
    wones = nc.dram_tensor("wones", [128, STRIPE], fp8, kind="ExternalInput").ap()
    outT = nc.dram_tensor("outT", [2, bl], f32, kind="ExternalOutput").ap()

    with tile.TileContext(nc) as tc, ExitStack() as ctx:
        wp_pool = ctx.enter_context(tc.tile_pool(name="wp", bufs=1))
        feat_pool = ctx.enter_context(tc.tile_pool(name="feat", bufs=12))
        x_pool = ctx.enter_context(tc.tile_pool(name="x", bufs=6))
        gw_pool = ctx.enter_context(tc.tile_pool(name="gw", bufs=4))
        grow_pool = ctx.enter_context(tc.tile_pool(name="grow", bufs=4))
        h_pool = ctx.enter_context(tc.tile_pool(name="h", bufs=10))
        sh_pool = ctx.enter_context(tc.tile_pool(name="sh", bufs=26))
        f_pool = ctx.enter_context(tc.tile_pool(name="f", bufs=2))
        pen_pool = ctx.enter_context(tc.tile_pool(name="pen", bufs=4))
        o_pool = ctx.enter_context(tc.tile_pool(name="o", bufs=4))
        # gb placed last: separates the GPSIMD broadcast-write region from
        # the h/sh regions the DVE muls read, reducing SBUF contention
        gb_pool = ctx.enter_context(tc.tile_pool(name="gb", bufs=6))

        px_pool = ctx.enter_context(tc.tile_pool(name="px", bufs=2, space="PSUM"))
        ph_pool = ctx.enter_context(tc.tile_pool(name="ph", bufs=2, space="PSUM"))
        pf_pool = ctx.enter_context(tc.tile_pool(name="pf", bufs=1, space="PSUM"))
        ps_pool = ctx.enter_context(tc.tile_pool(name="ps", bufs=1, space="PSUM"))

        # preload packed weights once. The two small tensors go first on
        # the sync ring to absorb the queue's cold first-transfer penalty
        # before the feature streams start; proj weights lead the scalar
        # ring so matmuls can start early.
        Bz = wp_pool.tile([128, WBCOLS], f32)
        nc.sync.dma_start(Bz[:], wbias[:])
        Wbf = wp_pool.tile([128, WBFCOLS], bf16)
        nc.sync.dma_start(Wbf[:], wmatbf[:])
        W8 = wp_pool.tile([128, W8COLS], fp8)
        nc.scalar.dma_start(W8[:, :OFF_W1], wmat8[:, :OFF_W1])
        nc.scalar.dma_start(W8[:, OFF_W1:], wmat8[:, OFF_W1:])

        def w8pair(off, m=128, parts=128):
            # stationary [K=128, 2, m] DoubleRow pair at col offset `off`
            return W8[:parts, off:off + 2 * m].rearrange(
                "p (two m) -> p two m", two=2
            )

        def w8s(off, n, parts=128):
            return W8[:parts, off:off + n]

        def wb(off, n, parts=128):
            return Wbf[:parts, off:off + n]

        def bslice(off, parts=128):
            return Bz[:parts, off:off + 1]

        featT_t = featT.rearrange("m (k p) b -> m p k b", p=128)

        pends = []  # (sh, gwT, bsl) of the previous three stripes
        head_pend = None  # (pen, bsl) awaiting its head matmul

        def emit_l2(pend):
            sh, gwT, bsl = pend
            pf = pf_pool.tile([128, STRIPE], f32, tag="pf")
            nc.tensor.matmul(
                pf[:], wb(OFF_B2B, 128, parts=NE), gwT[:],
                start=True, stop=False,
            )
            for e in range(NE):
                nc.tensor.matmul(
                    pf[:],
                    wb(OFF_W2B + e * 128, 128),
                    sh[e][:],
                    start=False,
                    stop=(e == NE - 1),
                )
            fT = f_pool.tile([128, STRIPE], bf16, tag="f")
            nc.scalar.copy(fT[:], pf[:])
            return fT

        def emit_pre(fT):
            pp = ps_pool.tile([64, STRIPE], f32, tag="ps")
            nc.tensor.matmul(pp[:], wb(OFF_PRE, 64), fT[:],
                             start=True, stop=True)
            pen = pen_pool.tile([64, STRIPE], bf16, tag="pen")
            nc.vector.tensor_scalar(
                pen[:], pp[:], bslice(OFF_PREB, parts=64), 0.0,
                op0=ALU.add, op1=ALU.max,
            )
            return pen

        def emit_head2(pen, bsl):
            po = ps_pool.tile([2, STRIPE], f32, tag="ps")
            nc.tensor.matmul(po[:], wb(OFF_HEAD, 2, parts=64), pen[:],
                             start=True, stop=True)
            ot = o_pool.tile([2, STRIPE], f32, tag="o")
            nc.scalar.activation(
                ot[:], po[:], AF.Identity, bias=bslice(OFF_HEADB, parts=2),
                scale=1.0,
            )
            nc.scalar.dma_start(outT[:, bsl], ot[:])

        for s in range(n_stripes):
            bsl = slice(s * STRIPE, (s + 1) * STRIPE)

            # ---- load features (fp8, 0.39 MB per modality) ----
            ft = []
            for m in range(NM):
                t = feat_pool.tile([128, KIN, STRIPE], fp8, tag="feat")
                # modality 1 rides the scalar ring: it idles after the
                # weight preload while sync serially delivers 3 tiles/stripe
                eng = nc.scalar if m == 1 else nc.sync
                eng.dma_start(t[:], featT_t[m, :, :, bsl])
                ft.append(t)

            # ---- per-modality projection -> xT chunks (fp8); 4th chunk
            # is all-ones (bias rider for the W1 DoubleRow passes) ----
            xt = x_pool.tile([128, KX + 1, STRIPE], fp8, tag="x")
            nc.scalar.dma_start(xt[:, KX, :], wones[:])
            for m in range(NM):
                px = px_pool.tile([128, STRIPE], f32, tag="px")
                for k in range(KIN // 2):
                    nc.tensor.matmul(
                        px[:],
                        w8pair(OFF_PROJ + m * KIN * 128 + k * 256),
                        ft[m][:, 2 * k:2 * k + 2, :],
                        start=(k == 0),
                        stop=(k == KIN // 2 - 1),
                        perf_mode=DR,
                    )
                nc.scalar.activation(
                    xt[:, m, :], px[:], AF.Identity,
                    bias=bslice(OFF_PROJB + m), scale=1.0 / WS,
                )

            # ---- finish head of an older stripe (pen ACT long done) ----
            if head_pend is not None:
                emit_head2(*head_pend)
                head_pend = None

            # ---- stage-2, three stripes back: l2 accumulation ----
            fT_prev = None
            if len(pends) == 2:
                p0 = pends.pop(0)
                fT_prev = emit_l2(p0)
                pend_bsl = p0[2]

            # ---- gate: softmax over 8 experts ----
            pg = ps_pool.tile([NE, STRIPE], f32, tag="ps")
            nc.tensor.matmul(
                pg[:], w8pair(OFF_GATE)[:, :, :NE], xt[:, 0:2, :],
                start=True, stop=False, perf_mode=DR,
            )
            nc.tensor.matmul(
                pg[:], w8s(OFF_GATE + 256, NE), xt[:, 2, :],
                start=False, stop=True,
            )
            eT = gw_pool.tile([NE, STRIPE], bf16, tag="eT")
            nc.scalar.activation(
                eT[:], pg[:], AF.Exp, bias=bslice(OFF_GATEB, parts=NE),
                scale=1.0 / WS,
            )
            psum_s = ps_pool.tile([NE, STRIPE], f32, tag="ps")
            nc.tensor.matmul(
                psum_s[:], wb(OFF_ONES, NE, parts=NE), eT[:],
                start=True, stop=True,
            )
            rT = gw_pool.tile([NE, STRIPE], f32, tag="rT")
            nc.vector.reciprocal_approx_fast(rT[:], psum_s[:])
            gwT = gw_pool.tile([NE, STRIPE], bf16, tag="gwT")
            nc.vector.tensor_mul(gwT[:], eT[:], rT[:])

            # gather gate rows onto partition 0; broadcast on idle GPSIMD
            grow = grow_pool.tile([1, NE, STRIPE], bf16, tag="grow")
            nc.scalar.dma_start(grow[:], gwT[:])

            # ---- experts: ph = W1.T x (+b1 via ones-rider DoubleRow);
            # h evictions merged across expert pairs (2-bank PSUM ACT);
            # sh = h * gw[e] on DVE from the GPSIMD broadcast ----
            sh = []
            for j in range(NE // 2):
                php = ph_pool.tile([128, 2, STRIPE], f32, tag="ph")
                for i in range(2):
                    e = 2 * j + i
                    off = OFF_W1 + e * 512
                    nc.tensor.matmul(
                        php[:, i, :], w8pair(off), xt[:, 0:2, :],
                        start=True, stop=False, perf_mode=DR,
                    )
                    nc.tensor.matmul(
                        php[:, i, :], w8pair(off + 256), xt[:, 2:4, :],
                        start=False, stop=True, perf_mode=DR,
                    )
                hp = h_pool.tile([128, 2, STRIPE], bf16, tag="h")
                nc.scalar.activation(hp[:], php[:], AF.Relu, scale=1.0 / WS)
                for i in range(2):
                    e = 2 * j + i
                    gb = gb_pool.tile([128, STRIPE], bf16, tag="gb")
                    nc.gpsimd.partition_broadcast(
                        gb[:], grow[0:1, e, :], channels=128
                    )
                    sht = sh_pool.tile([128, STRIPE], bf16, tag="sh")
                    nc.vector.tensor_mul(sht[:], hp[:, i, :], gb[:])
                    sh.append(sht)

            if fT_prev is not None:
                head_pend = (emit_pre(fT_prev), pend_bsl)
            pends.append((sh, gwT, bsl))

        if head_pend is not None:
            emit_head2(*head_pend)
        flush = [(emit_l2(p0), p0[2]) for p0 in pends]
        pens = [(emit_pre(fT), bsl_) for fT, bsl_ in flush]
        for pen, bsl_ in pens:
            emit_head2(pen, bsl_)

    nc.compile()
    return nc


_PROGRAM = None


def _get_program():
    global _PROGRAM
    if _PROGRAM is None:
        _PROGRAM = build_program()
    return _PROGRAM


def make_in_maps(inputs):
    """Host-side shard + layout prep: list of 8 per-core input maps."""
    w8, wbf, wbias = pack_weights(inputs)
    feats = [
        np.asarray(inputs["feat_text"], np.float32),
        np.asarray(inputs["feat_audio"], np.float32),
        np.asarray(inputs["feat_video"], np.float32),
    ]
    in_maps = []
    for c in range(NCORES):
        sl = slice(c * BL, (c + 1) * BL)
        featT = np.stack([np.ascontiguousarray(f[sl].T) for f in feats])
        in_maps.append({
            "featT": featT.astype(E4M3),
            "wmat8": w8,
            "wmatbf": wbf,
            "wbias": wbias,
            "wones": np.ones((128, STRIPE), np.float32).astype(E4M3),
        })
    return in_maps


def run_on_hw(inputs, trace=False):
    from concourse.bass_utils import run_bass_kernel_spmd

    nc = _get_program()
    in_maps = make_in_maps(inputs)
    res = run_bass_kernel_spmd(
        nc, in_maps, core_ids=list(range(NCORES)), trace=trace
    )
    out = np.concatenate([r["outT"].T for r in res.results], axis=0)
    return out, res


def kernel(**inputs):
    out, _ = run_on_hw(inputs, trace=False)
    return out
